# revision 3
# baseline (speedup 1.0000x reference)
"""Trainium2 Bass kernel for nn_CandidateConditionedSequenceDecoder.

Contract: kernel(**inputs) takes FULL unsharded inputs (as produced by
setup_inputs) and returns the FULL [B, D] output. Internally shards batch B
across 8 NeuronCores (2 batches/core); each core processes its 2 batches x 4
sequences = 8 independent (n, b) "pairs" and applies the final softmax gate
over N locally.

Self-contained: all shapes/sharding hardcoded.
"""
import sys
sys.path.insert(0, "/opt/trn_rl_repo")

import numpy as np
import concourse.bass as bass
import concourse.tile as tile
import concourse.mybir as mybir

f32 = mybir.dt.float32
f32r = mybir.dt.float32r
ALU = mybir.AluOpType
ACTF = mybir.ActivationFunctionType
AX = mybir.AxisListType

# problem constants
D = 512
H = 8
HD = 64
HID = 2048
R = 128
BLK = 32
K = 16
EPS = 1e-6
B = 16
S = 4096
N = 4
NCORES = 8
BPC = B // NCORES          # batches per core = 2
P = BPC * N                # pairs per core = 8


# this walrus build rejects instructions carrying >1 semaphore wait; split
# extras into standalone EventSemaphore instructions on the same engine.
def _split_multi_waits(nc, max_waits=1):
    n_split = 0
    for f in nc.m.functions:
        for bb in f.blocks:
            insts = list(bb.instructions)
            out = []
            changed = False
            for inst in insts:
                si = inst.sync_info
                if si is not None and len(si.on_wait) > max_waits:
                    waits = list(si.on_wait)
                    extra, keep = waits[:-max_waits], waits[-max_waits:]
                    for i, w in enumerate(extra):
                        ev = mybir.InstEventSemaphore(
                            name=f"{inst.name}_wsplit{i}", engine=inst.engine)
                        ev.sync_info = mybir.SyncInfo(on_wait=[w], on_update=[])
                        out.append(ev)
                        n_split += 1
                    inst.sync_info = mybir.SyncInfo(
                        on_wait=keep, on_update=list(si.on_update))
                    changed = True
                out.append(inst)
            if changed:
                try:
                    bb.instructions = out
                except Exception:
                    bb.instructions.clear()
                    bb.instructions.extend(out)
    return n_split


def _fix_range_clear(nc):
    """walrus rejects EVENT_SEMAPHORE_RANGE_CLEAR ("ISA wrong length");
    replace with per-semaphore write-0 EventSemaphore instructions."""
    import re
    n_fixed = 0
    for f in nc.m.functions:
        for bb in f.blocks:
            insts = list(bb.instructions)
            out = []
            changed = False
            for inst in insts:
                if (type(inst).__name__ == "InstISA"
                        and "EVENT_SEMAPHORE_RANGE_CLEAR" in inst.concise()):
                    m = re.search(r"range_first=(\d+) range_last=(\d+)",
                                  inst.concise())
                    first, last = int(m.group(1)), int(m.group(2))
                    for i, semid in enumerate(range(first, last + 1)):
                        ev = mybir.InstEventSemaphore(
                            name=f"{inst.name}_clr{i}", engine=inst.engine)
                        upd = mybir.SyncUpdate(sync_type="semaphore", id=semid,
                                               update_mode="sem-wr-imm",
                                               update_value=0)
                        waits = (list(inst.sync_info.on_wait)
                                 if (i == 0 and inst.sync_info) else [])
                        ev.sync_info = mybir.SyncInfo(on_wait=waits,
                                                      on_update=[upd])
                        out.append(ev)
                    changed = True
                    n_fixed += 1
                else:
                    out.append(inst)
            if changed:
                try:
                    bb.instructions = out
                except Exception:
                    bb.instructions.clear()
                    bb.instructions.extend(out)
    return n_fixed


def build(S_=S, P_=P, split=True, reps=1):
    """Build the per-core Bass program. Parameterized by sequence length for
    small-scale simulator testing; hardware uses S_=4096, P_=8. reps>1
    repeats the whole body inside one program (for slope-based timing)."""
    NB = S_ // BLK            # blocks per sequence (128 on hw)
    NCH = S_ // 512           # 512-token streaming groups (8 on hw)
    MT = 128 + NB + 2         # memory tokens: recent | blocks | (gtok, sink)
    assert NB <= 128 and S_ % 512 == 0

    nc = bass.Bass("TRN2", target_bir_lowering=False, debug=False)

    # register EPS as a const AP so ACT Sqrt can take it as bias
    _eps_t = nc.alloc_sbuf_tensor("const-float32-eps", [128, 1], f32)
    nc.gpsimd.memset(_eps_t.ap(), EPS)
    nc.const_aps.aps[(f32, EPS)] = _eps_t.ap()
    nc.all_engine_barrier()

    SEQ = nc.dram_tensor("seq", (P_, S_, D), f32r, kind="ExternalInput").ap()
    # packed f32r consts: [0:2NB] shifted block-selector, [2NB:2NB+128] identity,
    # col 2NB+128 ones, col 2NB+129 1/NB, row0 of [2NB+130 : 2NB+258] ones-row
    C0 = 2 * NB
    CONST = nc.dram_tensor("constr", (128, C0 + 131), f32r, kind="ExternalInput").ap()
    CONSTF = nc.dram_tensor("constf", (128, 256), f32, kind="ExternalInput").ap()
    QWB = nc.dram_tensor("qwb", (128, BPC * D), f32, kind="ExternalInput").ap()
    WLT = nc.dram_tensor("wlt", (128, BPC * 4 * H), f32r, kind="ExternalInput").ap()
    SINKN = nc.dram_tensor("sinkn", (1, D), f32r, kind="ExternalInput").ap()
    WVT = nc.dram_tensor("wvt", (128, 4 * D), f32r, kind="ExternalInput").ap()
    OWT = nc.dram_tensor("owt", (64, H * D), f32r, kind="ExternalInput").ap()
    GUT = nc.dram_tensor("gut", (128, 4 * 2 * HID), f32r, kind="ExternalInput").ap()
    DWT = nc.dram_tensor("dwt", (128, (HID // 128) * D), f32r, kind="ExternalInput").ap()
    B0C = nc.dram_tensor("b0c", (128, 4), f32, kind="ExternalInput").ap()
    GUB = nc.dram_tensor("gub", (128, 2 * HID // 128), f32, kind="ExternalInput").ap()
    DBC = nc.dram_tensor("dbc", (128, 4), f32, kind="ExternalInput").ap()
    GWA = nc.dram_tensor("gwa", (128, 4), f32r, kind="ExternalInput").ap()
    CQ = nc.dram_tensor("cq", (1, P_), f32, kind="ExternalInput").ap()
    OUT = nc.dram_tensor("out", (BPC, D), f32, kind="ExternalOutput").ap()

    with tile.TileContext(nc) as tc:
      import contextlib
      for _rep in range(reps):
        with contextlib.ExitStack() as ctx:
            cpool = ctx.enter_context(tc.tile_pool(name="consts", bufs=1))
            wpool = ctx.enter_context(tc.tile_pool(name="weights", bufs=1))
            spool = ctx.enter_context(tc.tile_pool(name="stream", bufs=3))
            ppool = ctx.enter_context(tc.tile_pool(name="pair", bufs=2))
            shpool = ctx.enter_context(tc.tile_pool(name="shared", bufs=1))
            ps_sm = ctx.enter_context(tc.tile_pool(name="ps_sm", bufs=2, space="PSUM"))
            pair_ctx = ctx.enter_context(contextlib.ExitStack())
            ps_blk = pair_ctx.enter_context(tc.tile_pool(name="ps_blk", bufs=2, space="PSUM"))
            ps_tr = pair_ctx.enter_context(tc.tile_pool(name="ps_tr", bufs=2, space="PSUM"))
            ps_lg = pair_ctx.enter_context(tc.tile_pool(name="ps_lg", bufs=1, space="PSUM"))
            ps_mx = pair_ctx.enter_context(tc.tile_pool(name="ps_mx", bufs=1, space="PSUM"))

            cr = cpool.tile([128, C0 + 131], f32r, tag="constr")
            nc.sync.dma_start(cr[:], CONST)
            cf = cpool.tile([128, 256], f32, tag="constf")
            nc.sync.dma_start(cf[:], CONSTF)
            qwb = cpool.tile([128, BPC * D], f32, tag="qwb")
            nc.sync.dma_start(qwb[:], QWB)
            wlt = cpool.tile([128, BPC * 4 * H], f32r, tag="wlt")
            nc.sync.dma_start(wlt[:], WLT)
            ONES = cr[:, C0 + 128:C0 + 129]          # [128,1] ones (f32r)
            ONES2 = cr[:, C0 + 128:C0 + 130]         # [128,2] ones (f32r)
            INVNB = cr[:, C0 + 130:C0 + 131]         # [128,1] 1/NB
            IDENT = cr[:, C0:C0 + 128]
            ONESROWF = cf[0:1, 128:256]              # [1,128] ones (f32)

            wvt = wpool.tile([128, 4 * D], f32r, tag="wvt")
            nc.sync.dma_start(wvt[:], WVT)
            owt = wpool.tile([64, H * D], f32r, tag="owt")
            nc.sync.dma_start(owt[:], OWT)
            gut = wpool.tile([128, 4 * 2 * HID], f32r, tag="gut")
            nc.sync.dma_start(gut[:], GUT)
            dwt = wpool.tile([128, (HID // 128) * D], f32r, tag="dwt")
            nc.sync.dma_start(dwt[:], DWT)
            b0c = wpool.tile([128, 4], f32, tag="b0c")
            nc.sync.dma_start(b0c[:], B0C)
            gub = wpool.tile([128, 2 * HID // 128], f32, tag="gub")
            nc.sync.dma_start(gub[:], GUB)
            dbc = wpool.tile([128, 4], f32, tag="dbc")
            nc.sync.dma_start(dbc[:], DBC)
            gwa = wpool.tile([128, 4], f32r, tag="gwa")
            nc.sync.dma_start(gwa[:], GWA)
            cqt = wpool.tile([1, P_], f32, tag="cq")
            nc.sync.dma_start(cqt[:], CQ)

            scol = shpool.tile([NB, P_], f32, tag="scol")       # block scores
            mixT = shpool.tile([128, 4 * P_ * H], f32r, tag="mixT")
            upds = []   # [128,8] f32r updates per d-chunk (set in tail)

            # ---------------- per-pair phase ----------------
            for p in range(P_):
                b = p // N
                # stream S_ tokens, reduce 32-token blocks into PSUM via PE
                bps = ps_blk.tile([NB, D], f32, tag="blk")
                for g in range(NCH):
                    st = spool.tile([128, 2048], f32r, tag="st")
                    nc.sync.dma_start(
                        st[:].rearrange("k (c d) -> k c d", c=4),
                        SEQ[p, g * 512:(g + 1) * 512, :].rearrange(
                            "(c k) d -> k c d", c=4))
                    for c in range(4):
                        i = g * 4 + c
                        nc.tensor.matmul(
                            bps[:], cr[:, NB - 4 * i:2 * NB - 4 * i],
                            st[:, c * 512:(c + 1) * 512],
                            start=(i == 0), stop=(i == NCH * 4 - 1))

                # raw recent tokens + block means in SBUF (f32r)
                rec_r = ppool.tile([128, D], f32r, tag="rec_r")
                nc.sync.dma_start(rec_r[:], SEQ[p, S_ - R:S_, :])
                blk_r = ppool.tile([NB, D], f32r, tag="blk_r")
                nc.scalar.copy(blk_r[:], bps[:])

                # global token = mean of block means
                gps = ps_sm.tile([1, D], f32, tag="sm")
                nc.tensor.matmul(gps[:], INVNB[0:NB, :], blk_r[:],
                                 start=True, stop=True)
                xg = ppool.tile([1, D], f32r, tag="xg")
                nc.scalar.copy(xg[:], gps[:])

                # rms stats (mean-square -> sqrt -> reciprocal)
                scr = ppool.tile([128, D], f32, tag="scr")
                ms_r = ppool.tile([128, 1], f32, tag="ms_r")
                nc.scalar.activation(scr[:], rec_r[:], ACTF.Square,
                                     accum_out=ms_r[:])
                ms_b = ppool.tile([NB, 1], f32, tag="ms_b")
                nc.scalar.activation(scr[0:NB, :], blk_r[:], ACTF.Square,
                                     accum_out=ms_b[:])
                ms_g = ppool.tile([1, 1], f32, tag="ms_g")
                nc.scalar.activation(scr[0:1, :], xg[:], ACTF.Square,
                                     accum_out=ms_g[:])
                rq_r = ppool.tile([128, 1], f32, tag="rq_r")
                nc.scalar.activation(rq_r[:], ms_r[:], ACTF.Sqrt,
                                     bias=EPS, scale=1.0 / D)
                rq_b = ppool.tile([NB, 1], f32, tag="rq_b")
                nc.scalar.activation(rq_b[:], ms_b[:], ACTF.Sqrt,
                                     bias=EPS, scale=1.0 / D)
                rq_g = ppool.tile([1, 1], f32, tag="rq_g")
                nc.scalar.activation(rq_g[:], ms_g[:], ACTF.Sqrt,
                                     bias=EPS, scale=1.0 / D)
                ri_r = ppool.tile([128, 1], f32, tag="ri_r")
                nc.vector.reciprocal(ri_r[:], rq_r[:])
                ri_b = ppool.tile([NB, 1], f32, tag="ri_b")
                nc.vector.reciprocal(ri_b[:], rq_b[:])
                ri_g = ppool.tile([1, 1], f32, tag="ri_g")
                nc.vector.reciprocal(ri_g[:], rq_g[:])

                # normalized memory rows (token-on-partition, f32r)
                rec_n = ppool.tile([128, D], f32r, tag="rec_n")
                nc.vector.tensor_scalar(rec_n[:], rec_r[:], ri_r[:], None,
                                        op0=ALU.mult)
                blk_n = ppool.tile([NB, D], f32r, tag="blk_n")
                nc.vector.tensor_scalar(blk_n[:], blk_r[:], ri_b[:], None,
                                        op0=ALU.mult)
                xn = ppool.tile([2, D], f32r, tag="xn")
                nc.vector.tensor_scalar(xn[0:1, :], xg[:], ri_g[:], None,
                                        op0=ALU.mult)
                nc.sync.dma_start(xn[1:2, :], SINKN)

                # block scores = (blk_n . qw_b), qw_b = mn_w * qn_query[b]
                sc_raw = ppool.tile([NB, 1], f32, tag="sc_raw")
                nc.vector.tensor_tensor(out=scr[0:NB, :], in0=blk_n[:],
                                        in1=qwb[0:NB, b * D:(b + 1) * D],
                                        op=ALU.mult)
                nc.vector.reduce_sum(sc_raw[:], scr[0:NB, :], axis=AX.X)
                nc.vector.tensor_copy(scol[:, p:p + 1], sc_raw[:])

                # transpose memory into d-on-partition layout for logits
                memT = ppool.tile([128, 4 * MT], f32r, tag="memT")
                for c in range(4):
                    pt = ps_tr.tile([128, 128], f32r, tag="tr")
                    nc.tensor.transpose(pt[:], rec_n[:, c * 128:(c + 1) * 128],
                                        IDENT)
                    nc.vector.tensor_copy(memT[:, c * MT:c * MT + 128], pt[:])
                    pt2 = ps_tr.tile([128, 128], f32r, tag="tr")
                    nc.tensor.transpose(pt2[0:128, 0:NB],
                                        blk_n[:, c * 128:(c + 1) * 128],
                                        IDENT[0:NB, 0:NB])
                    nc.vector.tensor_copy(
                        memT[:, c * MT + 128:c * MT + 128 + NB],
                        pt2[0:128, 0:NB])
                    pt3 = ps_tr.tile([128, 128], f32r, tag="tr")
                    nc.tensor.transpose(pt3[0:128, 0:2],
                                        xn[:, c * 128:(c + 1) * 128],
                                        IDENT[0:2, 0:2])
                    nc.vector.tensor_copy(
                        memT[:, c * MT + 128 + NB:(c + 1) * MT],
                        pt3[0:128, 0:2])

                # attention logits [H, MT] (scale & mn_w folded into wlt)
                lg = ps_lg.tile([H, MT], f32, tag="lg")
                for c in range(4):
                    nc.tensor.matmul(lg[:], wlt[:, (b * 4 + c) * H:(b * 4 + c + 1) * H],
                                     memT[:, c * MT:(c + 1) * MT],
                                     start=(c == 0), stop=(c == 3))

                # top-16 threshold over block scores
                stp = ps_sm.tile([1, NB], f32, tag="sm")
                nc.tensor.transpose(stp[:], scol[:, p:p + 1], cf[0:NB, 0:NB])
                srow = ppool.tile([1, NB], f32, tag="srow")
                nc.vector.tensor_copy(srow[:], stp[:])
                m1 = ppool.tile([1, 8], f32, tag="m1")
                nc.vector.max(out=m1[:], in_=srow[:])
                srow2 = ppool.tile([1, NB], f32, tag="srow2")
                nc.vector.match_replace(out=srow2[:], in_to_replace=m1[:],
                                        in_values=srow[:], imm_value=-1e30)
                m2 = ppool.tile([1, 8], f32, tag="m2")
                nc.vector.max(out=m2[:], in_=srow2[:])
                tbp = ps_sm.tile([NB, 1], f32, tag="sm")
                nc.tensor.matmul(tbp[:], ONESROWF[:, 0:NB], m2[:, 7:8],
                                 start=True, stop=True)
                t16s = ppool.tile([NB, 1], f32, tag="t16s")
                nc.vector.tensor_copy(t16s[:], tbp[:])
                selc = ppool.tile([NB, 1], f32, tag="selc")
                nc.vector.tensor_scalar(selc[:], scol[:, p:p + 1], t16s[:],
                                        None, op0=ALU.is_ge)

                # softmax numerator: e = exp(logits - rowmax), f32r for PE
                rmax = ppool.tile([H, 1], f32, tag="rmax")
                nc.vector.reduce_max(rmax[:], lg[:], axis=AX.X)
                rmn = ppool.tile([H, 1], f32, tag="rmn")
                nc.vector.tensor_scalar(rmn[:], rmax[:], -1.0, None,
                                        op0=ALU.mult)
                e_sb = ppool.tile([H, MT], f32r, tag="e_sb")
                nc.scalar.activation(e_sb[:], lg[:], ACTF.Exp, bias=rmn[:],
                                     scale=1.0)

                # transpose attn weights to token-on-partition; mask blocks
                eT_r = ppool.tile([128, H], f32r, tag="eT_r")
                pt = ps_tr.tile([128, 128], f32r, tag="tr")
                nc.tensor.transpose(pt[0:128, 0:H], e_sb[:, 0:128],
                                    IDENT[0:H, 0:H])
                nc.vector.tensor_copy(eT_r[:], pt[0:128, 0:H])
                eT_b = ppool.tile([NB, H], f32r, tag="eT_b")
                pt = ps_tr.tile([128, 128], f32r, tag="tr")
                nc.tensor.transpose(pt[0:NB, 0:H], e_sb[:, 128:128 + NB],
                                    IDENT[0:H, 0:H])
                nc.vector.tensor_scalar(eT_b[:], pt[0:NB, 0:H], selc[:], None,
                                        op0=ALU.mult)
                eT_x = ppool.tile([2, H], f32r, tag="eT_x")
                pt = ps_tr.tile([128, 128], f32r, tag="tr")
                nc.tensor.transpose(pt[0:2, 0:H], e_sb[:, 128 + NB:MT],
                                    IDENT[0:H, 0:H])
                nc.vector.tensor_copy(eT_x[:], pt[0:2, 0:H])

                # mix[h] = sum_tok attn * mem_n ; denom = sum_tok attn
                mxp = ps_mx.tile([H, D], f32, tag="mix")
                nc.tensor.matmul(mxp[:], eT_r[:], rec_n[:], start=True, stop=False)
                nc.tensor.matmul(mxp[:], eT_b[:], blk_n[:], start=False, stop=False)
                nc.tensor.matmul(mxp[:], eT_x[:], xn[:], start=False, stop=True)
                dnp = ps_sm.tile([H, 2], f32, tag="sm")
                nc.tensor.matmul(dnp[:], eT_r[:], ONES2, start=True, stop=False)
                nc.tensor.matmul(dnp[:], eT_b[:], ONES2[0:NB, :], start=False, stop=False)
                nc.tensor.matmul(dnp[:], eT_x[:], ONES2[0:2, :], start=False, stop=True)
                rd = ppool.tile([H, 1], f32, tag="rd")
                nc.vector.reciprocal(rd[:], dnp[:, 0:1])
                mixn = ppool.tile([H, D], f32r, tag="mixn")
                nc.vector.tensor_scalar(mixn[:], mxp[:], rd[:], None,
                                        op0=ALU.mult)
                for c in range(4):
                    pt = ps_tr.tile([128, 128], f32r, tag="tr")
                    nc.tensor.transpose(pt[0:128, 0:H],
                                        mixn[:, c * 128:(c + 1) * 128],
                                        IDENT[0:H, 0:H])
                    nc.vector.tensor_copy(
                        mixT[:, c * P_ * H + p * H:c * P_ * H + (p + 1) * H],
                        pt[0:128, 0:H])

            # ---------------- batched tail over all pairs ----------------
            pair_ctx.close()
            ps_ctx = ctx.enter_context(tc.tile_pool(name="ps_ctx", bufs=1, space="PSUM"))
            ps_att = ctx.enter_context(tc.tile_pool(name="ps_att", bufs=1, space="PSUM"))
            ps_gv = ctx.enter_context(tc.tile_pool(name="ps_gv", bufs=3, space="PSUM"))
            tail = ctx.enter_context(tc.tile_pool(name="tail", bufs=1))

            # ctx = blockdiag(Wv') @ mix  (per head), batched over pairs
            ctx_sb = []
            for h in range(H):          # one psum group per head
                cps = ps_ctx.tile([64, P_], f32, tag="ctx")
                for c in range(4):
                    nc.tensor.matmul(
                        cps[:],
                        wvt[:, c * D + h * 64:c * D + (h + 1) * 64],
                        mixT[:, c * P_ * H + h:(c + 1) * P_ * H:H],
                        start=(c == 0), stop=(c == 3))
                csb = tail.tile([64, P_], f32r, tag=f"ctxh{h}")
                nc.vector.tensor_copy(csb[:], cps[:])
                ctx_sb.append(csb)

            # attended = out_w @ ctx + b0
            att_sb = []
            sq_sb = []
            for m in range(4):
                aps = ps_att.tile([128, P_], f32, tag="att")
                for h in range(H):
                    nc.tensor.matmul(aps[:],
                                     owt[0:64, h * D + m * 128:h * D + (m + 1) * 128],
                                     ctx_sb[h][:], start=(h == 0), stop=(h == H - 1))
                asb = tail.tile([128, P_], f32, tag=f"att{m}")
                nc.vector.tensor_scalar(asb[:], aps[:], b0c[:, m:m + 1], None,
                                        op0=ALU.add)
                att_sb.append(asb)
                sq = tail.tile([128, P_], f32r, tag=f"sq{m}")
                nc.scalar.activation(sq[:], asb[:], ACTF.Square)
                sq_sb.append(sq)

            # rms over d (partition dim) via ones-matmul
            ssp = ps_sm.tile([1, P_], f32, tag="sm")
            for m in range(4):
                nc.tensor.matmul(ssp[:], ONES, sq_sb[m][:], start=(m == 0),
                                 stop=(m == 3))
            sss = tail.tile([1, P_], f32, tag="sss")
            nc.scalar.activation(sss[:], ssp[:], ACTF.Sqrt, bias=EPS,
                                 scale=1.0 / D)
            rin = tail.tile([1, P_], f32, tag="rin")
            nc.vector.reciprocal(rin[:], sss[:])
            rbp = ps_sm.tile([128, P_], f32, tag="sm")
            nc.tensor.matmul(rbp[:], ONESROWF, rin[:], start=True, stop=True)
            h_sb = []
            for m in range(4):
                hs = tail.tile([128, P_], f32r, tag=f"h{m}")
                nc.vector.tensor_tensor(out=hs[:], in0=att_sb[m][:], in1=rbp[:],
                                        op=ALU.mult)
                h_sb.append(hs)

            # SwiGLU: gu' = [gate|value] chunks; sv = silu(g+bg)*(v+bv)
            sv_all = tail.tile([128, (HID // 128) * P_], f32r, tag="sv")
            for i in range(HID // 128):
                gps_ = ps_gv.tile([128, P_], f32, tag="gv")
                vps_ = ps_gv.tile([128, P_], f32, tag="gv")
                for c in range(4):
                    nc.tensor.matmul(gps_[:],
                                     gut[:, c * 2 * HID + i * 128:c * 2 * HID + (i + 1) * 128],
                                     h_sb[c][:], start=(c == 0), stop=(c == 3))
                for c in range(4):
                    nc.tensor.matmul(vps_[:],
                                     gut[:, c * 2 * HID + HID + i * 128:c * 2 * HID + HID + (i + 1) * 128],
                                     h_sb[c][:], start=(c == 0), stop=(c == 3))
                # silu(x) = x * sigmoid(x), with x = g + bias (CoreSim has no
                # native Silu; the decomposition matches it closely)
                sgm = tail.tile([128, P_], f32, tag="sgm")
                nc.scalar.activation(sgm[:], gps_[:], ACTF.Sigmoid,
                                     bias=gub[:, i:i + 1], scale=1.0)
                sg = tail.tile([128, P_], f32, tag="sg")
                nc.vector.scalar_tensor_tensor(
                    out=sg[:], in0=gps_[:], scalar=gub[:, i:i + 1],
                    in1=sgm[:], op0=ALU.add, op1=ALU.mult)
                nc.vector.scalar_tensor_tensor(
                    out=sv_all[:, i * P_:(i + 1) * P_], in0=vps_[:],
                    scalar=gub[:, (HID // 128) + i:(HID // 128) + i + 1],
                    in1=sg[:], op0=ALU.add, op1=ALU.mult)

            # down proj + residual
            upd_sb = []
            for m in range(4):
                fps = ps_att.tile([128, P_], f32, tag="f")
                for k in range(HID // 128):
                    nc.tensor.matmul(fps[:],
                                     dwt[:, k * D + m * 128:k * D + (m + 1) * 128],
                                     sv_all[:, k * P_:(k + 1) * P_],
                                     start=(k == 0), stop=(k == HID // 128 - 1))
                usb = tail.tile([128, P_], f32r, tag=f"upd{m}")
                nc.vector.scalar_tensor_tensor(
                    out=usb[:], in0=fps[:], scalar=dbc[:, m:m + 1],
                    in1=att_sb[m][:], op0=ALU.add, op1=ALU.add)
                upd_sb.append(usb)

            # gate score = gwa . upd + (host query part)
            scp = ps_sm.tile([1, P_], f32, tag="sm")
            for m in range(4):
                nc.tensor.matmul(scp[:], gwa[:, m:m + 1], upd_sb[m][:],
                                 start=(m == 0), stop=(m == 3))
            scs = tail.tile([1, P_], f32, tag="scs")
            nc.vector.tensor_tensor(out=scs[:], in0=scp[:], in1=cqt[:],
                                    op=ALU.add)
            # softmax over n within each batch group of 4 (logits are O(1),
            # exp without max-subtraction is safe)
            e8 = tail.tile([1, P_], f32, tag="e8")
            nc.scalar.activation(e8[:], scs[:], ACTF.Exp)
            den2 = tail.tile([1, BPC], f32, tag="den2")
            nc.vector.reduce_sum(den2[:],
                                 e8[:].rearrange("a (b n) -> a b n", n=N),
                                 axis=AX.X)
            r2 = tail.tile([1, BPC], f32, tag="r2")
            nc.vector.reciprocal(r2[:], den2[:])
            ebp = ps_sm.tile([128, P_], f32, tag="sm")
            nc.tensor.matmul(ebp[:], ONESROWF, e8[:], start=True, stop=True)
            rbp2 = ps_sm.tile([128, BPC], f32, tag="sm")
            nc.tensor.matmul(rbp2[:], ONESROWF, r2[:], start=True, stop=True)

            for m in range(4):
                wu = tail.tile([128, P_], f32, tag="wu")
                nc.vector.tensor_tensor(out=wu[:], in0=upd_sb[m][:],
                                        in1=ebp[:], op=ALU.mult)
                oun = tail.tile([128, BPC], f32, tag="oun")
                nc.vector.reduce_sum(oun[:],
                                     wu[:].rearrange("a (b n) -> a b n", n=N),
                                     axis=AX.X)
                oc = tail.tile([128, BPC], f32, tag="oc")
                nc.vector.tensor_tensor(out=oc[:], in0=oun[:], in1=rbp2[:],
                                        op=ALU.mult)
                nc.sync.dma_start(
                    OUT[:, m * 128:(m + 1) * 128].transpose([1, 0]), oc[:])

    if split:
        _split_multi_waits(nc)
        _fix_range_clear(nc)
    return nc


def host_prep(query, sequences, masks=None, qn_w=None, mn_w=None,
              ffn_norm_w=None, sink=None, in_proj_w=None, in_proj_b=None,
              out_w=None, out_b=None, gate_w=None, gate_b=None,
              gu_w=None, gu_b=None, down_w=None, down_b=None,
              S_=S, ncores=NCORES):
    """Fold all small parameter math on host; build per-core input maps."""
    NB = S_ // BLK
    C0 = 2 * NB
    f = np.float32
    query = np.asarray(query, f)
    sequences = np.asarray(sequences, f)
    scale = 1.0 / np.sqrt(HD)

    qn = query * (1.0 / np.sqrt(np.mean(query.astype(np.float64) ** 2, axis=-1,
                                        keepdims=True) + EPS))
    qn = (qn * qn_w).astype(f)                     # [B, D] rms'd query
    Wq, Wk, Wv = in_proj_w[0:D], in_proj_w[D:2 * D], in_proj_w[2 * D:3 * D]
    bq, bk, bv = in_proj_b[0:D], in_proj_b[D:2 * D], in_proj_b[2 * D:3 * D]
    qp = (qn @ np.asarray(Wq, f).T + np.asarray(bq, f))   # [B, D]

    # logits weight per (b, h): scale * (Wk_h^T qp_bh) * mn_w  -> [B, D, H]
    WL = np.empty((query.shape[0], D, H), f)
    for h in range(H):
        Wkh = np.asarray(Wk[h * HD:(h + 1) * HD], f)      # [64, D]
        WL[:, :, h] = (qp[:, h * HD:(h + 1) * HD] @ Wkh) * np.asarray(mn_w, f) * scale

    WvT = (np.asarray(Wv, f) * np.asarray(mn_w, f)[None, :]).T.copy()  # [D, D(he)]
    OWTm = np.asarray(out_w, f).T.copy()                  # [he, do]
    b0 = (np.asarray(out_w, f) @ np.asarray(bv, f) + np.asarray(out_b, f))
    GUTm = (np.asarray(gu_w, f) * np.asarray(ffn_norm_w, f)[None, :]).T.copy()
    DWTm = np.asarray(down_w, f).T.copy()                 # [hid, do]

    sinkv = np.asarray(sink, f).reshape(-1)
    sink_n = (sinkv / np.sqrt(np.mean(sinkv.astype(np.float64) ** 2) + EPS)).astype(f)

    # packed f32r const block
    Smat = np.zeros((128, C0), f)
    for k in range(128):
        Smat[k, NB + k // 32] = 1.0 / 32.0
    constr = np.zeros((128, C0 + 131), f)
    constr[:, 0:C0] = Smat
    constr[:, C0:C0 + 128] = np.eye(128, dtype=f)
    constr[:, C0 + 128] = 1.0
    constr[:, C0 + 129] = 1.0
    constr[:, C0 + 130] = 1.0 / NB
    constf = np.zeros((128, 256), f)
    constf[:, 0:128] = np.eye(128, dtype=f)
    constf[0, 128:256] = 1.0

    gq = np.asarray(gate_w, f)[0, 0:D]
    ga = np.asarray(gate_w, f)[0, D:2 * D]
    gb = float(np.asarray(gate_b, f)[0])

    def chunk_cols(Mt):   # [d, cols] -> [128, 4*cols], rows d-chunked
        d, ncol = Mt.shape
        assert d == 4 * 128
        return np.concatenate([Mt[c * 128:(c + 1) * 128, :] for c in range(4)],
                              axis=1).astype(f)

    wvt_t = chunk_cols(WvT)           # [128, 4*512]
    owt_t = np.concatenate([OWTm[h * HD:(h + 1) * HD, :]
                            for h in range(H)], axis=1).astype(f)
    gut_t = chunk_cols(GUTm)          # [128, 4*4096]
    dwt_t = np.concatenate([DWTm[k * 128:(k + 1) * 128, :]
                            for k in range(HID // 128)], axis=1).astype(f)
    b0c = b0.reshape(4, 128).T.copy()
    gub_t = np.asarray(gu_b, f).reshape(2 * HID // 128, 128).T.copy()
    dbc = np.asarray(down_b, f).reshape(4, 128).T.copy()
    gwa_t = ga.reshape(4, 128).T.copy()

    in_maps = []
    for core in range(ncores):
        bs = range(core * BPC, (core + 1) * BPC)
        seq_c = np.ascontiguousarray(
            sequences[:, core * BPC:(core + 1) * BPC].transpose(1, 0, 2, 3)
            .reshape(BPC * N, S_, D))
        qwb = np.zeros((128, BPC * D), f)
        wlt = np.zeros((128, BPC * 4 * H), f)
        cq = np.zeros((1, BPC * N), f)
        for bl, bg in enumerate(bs):
            qwb[:, bl * D:(bl + 1) * D] = (np.asarray(mn_w, f) * qn[bg])[None, :]
            for c in range(4):
                wlt[:, (bl * 4 + c) * H:(bl * 4 + c + 1) * H] = \
                    WL[bg, c * 128:(c + 1) * 128, :]
            for n in range(N):
                cq[0, bl * N + n] = float(gq @ query[bg]) + gb
        in_maps.append({
            "seq": seq_c, "constr": constr, "constf": constf, "qwb": qwb,
            "wlt": wlt, "sinkn": sink_n.reshape(1, D), "wvt": wvt_t,
            "owt": owt_t, "gut": gut_t, "dwt": dwt_t, "b0c": b0c,
            "gub": gub_t, "dbc": dbc, "gwa": gwa_t, "cq": cq,
        })
    return in_maps


_NC_CACHE = {}


def _run(inputs, trace=False, **trace_kwargs):
    from concourse.bass_utils import run_bass_kernel_spmd
    masks = np.asarray(inputs["masks"])
    assert not masks.any(), "kernel fast path assumes all-false masks"
    if "hw" not in _NC_CACHE:
        _NC_CACHE["hw"] = build(S_=S, P_=P)
    nc = _NC_CACHE["hw"]
    in_maps = host_prep(**{k: v for k, v in inputs.items() if k != "masks"})
    res = run_bass_kernel_spmd(nc, in_maps, list(range(NCORES)), trace=trace,
                               **trace_kwargs)
    out = np.concatenate([res.results[i]["out"] for i in range(NCORES)], axis=0)
    return out.astype(np.float32), res


def kernel(**inputs):
    out, _ = _run(inputs, trace=False)
    return out

